# revision 69
# baseline (speedup 1.0000x reference)
"""Trainium2 Bass kernel for nn_CFAConv (cross-feature attention + conv block).

Self-contained: takes full unsharded inputs, shards (batch, image-half) across
8 NeuronCores, runs one SPMD Bass/Tile NEFF, and combines partial results on
the host.

Math (validated against the jax reference in numpy):
  x1 = w_q@in0 + b_q ; x2 = w_k@in0 + b_k ; x3 = w_v@in1 + b_v  (1x1 convs)
  aff = softmax_j(x2^T x3) ; x0 = x1 @ aff
  x0' = gelu(bn0(w_o@x0 + b_o))
  y = gelu(bn(conv3x3(concat(x0', in0)))) ; y = gelu(bn(conv3x3(y)))
  out = max_spatial(y + x0')
On-device simplifications:
  - softmax over j is invariant to per-column shifts => b_k drops entirely
  - x2^T(x3 + b_v) = x2^T x3 + (x2^T b_v) 1^T    => fold b_v into x3
  - (x1 + b_q 1^T) @ aff = x1@aff + b_q 1^T (aff columns sum to 1)
    => fold w_o@b_q into the out-projection bias (host-side)
  - eval-mode BN folds to per-channel scale/bias, fused into the gelu ACT op
  - softmax normalization deferred past the x1@exp(S) matmul (divide x0 by
    column sums); sums via a 5-level bf16 DVE pre-sum tree + one ones-matmul
  - no max-subtraction in softmax: |S| <= ~60 here; exp fits fp32 (max ~e88)
Precision: bf16 operands with fp32 PSUM accumulation for the attention path;
the two 3x3 convs run in fp8e4m3 with DoubleRow perf mode (2 contraction
tiles per pass at 0.5 cycles/row):
  - conv0 x0'-half: weights + acts naive fp8 (x0' is small vs in0 => cheap)
  - conv0 in0-half: weights hi+lo fp8 split, in0 hi+lo fp8 split (host-side),
    3-term product (Wh Xh + Wh Xl + Wl Xh)
  - conv1: weights hi+lo (host), c0 hi+lo split on DVE, 3-term
  (numpy bit-model: 1.3e-2 final rel err vs the 2e-2 budget)
Sharding: 8 cores = (4 batches) x (top/bottom image half). Each core computes
a 34-row window (32 owned + halo) so the two 3x3 convs need no communication;
per-row maxes [256, 34] go to the host which slices owned rows and reduces.
"""

from contextlib import ExitStack

import ml_dtypes
import numpy as np

import concourse.bass as bass
import concourse.tile as tile
from concourse import bacc, mybir
from concourse.bass_utils import run_bass_kernel_spmd

B, C, H, W = 4, 256, 64, 64
Ch = C // 2          # 128
N = H * W            # 4096
ROWS = 34            # per-core row window (32 owned + 2 halo)
KW = ROWS * W        # 2176 window positions
EPS = 1e-5

F32 = mybir.dt.float32
BF16 = mybir.dt.bfloat16
F8 = mybir.dt.float8e4
AF = mybir.ActivationFunctionType
AX = mybir.AxisListType
DR = mybir.MatmulPerfMode.DoubleRow
BF16NP = ml_dtypes.bfloat16
F8NP = ml_dtypes.float8_e4m3

# attention k-tiles over the 2176-column window
K_TILES = [(0, 512), (512, 512), (1024, 512), (1536, 512), (2048, 128)]
# conv output row-tiles (local rows 1..34 of the 36-row padded buffer)
ROW_TILES = [(1, 8), (9, 8), (17, 8), (25, 8), (33, 2)]

_CACHED = {}


def build_program():
    nc = bacc.Bacc("TRN2", target_bir_lowering=False, debug=False)

    def din(name, shape, dt=F32):
        return nc.dram_tensor(name, shape, dt, kind="ExternalInput").ap()

    in0b_d = din("in0b", [C, N], BF16)
    in1b_d = din("in1b", [C, KW], BF16)
    # in0 conv window, fp8 hi/lo, pre-padded to 66 cols (zero side columns)
    in0h_d = din("in0h", [C, ROWS * 66], F8)
    in0l_d = din("in0l", [C, ROWS * 66], F8)
    wq_t = din("wq_t", [C, Ch], BF16)     # (c, i)
    wk_t = din("wk_t", [C, Ch], BF16)
    wv_t = din("wv_t", [C, Ch], BF16)
    wo_t = din("wo_t", [Ch, C], BF16)     # (i, o)
    bv = din("bv", [Ch, 1])
    bias6_d = din("bias6", [6, C, 1])     # ao, bo, a0, b0, a1, b1
    # conv0 weights: [x0-half naive, in0-half hi, in0-half lo] (tap, ci, o)
    w0all_d = din("w0all", [3, 9, C, C], F8)
    w1all_d = din("w1all", [2, 9, C, C], F8)  # [hi, lo]
    idm_d = din("idm", [128, 128], BF16)
    out = nc.dram_tensor("out", [C, ROWS], F32, kind="ExternalOutput").ap()

    with tile.TileContext(nc) as tc, ExitStack() as ctx:
        persist = ctx.enter_context(tc.tile_pool(name="persist", bufs=1))
        psum = ctx.enter_context(tc.tile_pool(name="psum", bufs=2, space="PSUM"))
        psum1 = ctx.enter_context(tc.tile_pool(name="psum1", bufs=2, space="PSUM"))
        small = ctx.enter_context(tc.tile_pool(name="small", bufs=3))

        # ---- inputs: bf16 quarters of in0 (one DMA each: per-slice deps
        # because Tile dependencies are whole-tile). Weights + a small first
        # slice of in0 go first so the first matmul starts ASAP. ----
        # DMA issue costs ~1.26us/queue: strictly alternate the projection
        # inputs across the SP and ACT HWDGE queues in consumption order
        in0ap = in0b_d.rearrange("(a p) n -> p a n", a=2)
        in0q0a = persist.tile([128, 2, 128], BF16, tag="in0q0a")
        nc.scalar.dma_start(out=in0q0a, in_=in0ap[:, :, 0:128])
        wk_s = persist.tile([128, 2, Ch], BF16, tag="wk")
        nc.sync.dma_start(out=wk_s, in_=wk_t.rearrange("(a p) n -> p a n", a=2))
        in0q0b = persist.tile([128, 2, 384], BF16, tag="in0q0b")
        nc.scalar.dma_start(out=in0q0b, in_=in0ap[:, :, 128:512])
        wq_s = persist.tile([128, 2, Ch], BF16, tag="wq")
        nc.sync.dma_start(out=wq_s, in_=wq_t.rearrange("(a p) n -> p a n", a=2))
        in0q = [None] + [persist.tile([128, 2, 512], BF16, tag=f"in0q{q}",
                                      name=f"in0q{q}") for q in range(1, 8)]
        in1q = [persist.tile([128, 2, 1152], BF16, tag=f"in1q{q}",
                             name=f"in1q{q}") for q in range(2)]
        for q in range(1, 8):
            eng = nc.scalar if q % 2 == 0 else nc.sync
            eng.dma_start(out=in0q[q], in_=in0ap[:, :, q * 512:(q + 1) * 512])
        nc.sync.dma_start(
            out=in1q[0][:, :, :1024],
            in_=in1b_d.rearrange("(a p) n -> p a n", a=2)[:, :, 0:1024])
        nc.scalar.dma_start(
            out=in1q[1],
            in_=in1b_d.rearrange("(a p) n -> p a n", a=2)[:, :, 1024:KW])
        wv_s = persist.tile([128, 2, Ch], BF16, tag="wv")
        nc.scalar.dma_start(out=wv_s,
                            in_=wv_t.rearrange("(a p) n -> p a n", a=2))
        bv_s = persist.tile([128, 1], F32, tag="bv")
        nc.scalar.dma_start(out=bv_s, in_=bv)
        wo_s = persist.tile([128, C], BF16, tag="wo")
        nc.sync.dma_start(out=wo_s, in_=wo_t)
        bias_s = persist.tile([128, 12], F32, tag="bias6")
        nc.sync.dma_start(out=bias_s,
                          in_=bias6_d.rearrange("t (a p) o -> p (t a o)", a=2))
        ao_s, bo_s = bias_s[:, 0:2], bias_s[:, 2:4]
        a0_s, b0_s = bias_s[:, 4:6], bias_s[:, 6:8]
        a1_s, b1_s = bias_s[:, 8:10], bias_s[:, 10:12]
        ones_s = persist.tile([128, 1], BF16, tag="ones")
        nc.vector.memset(ones_s, 1.0)
        ones_row = persist.tile([1, 128], BF16, tag="ones_row")
        nc.vector.memset(ones_row, 1.0)
        id_s = persist.tile([128, 128], BF16, tag="idm")
        nc.sync.dma_start(out=id_s, in_=idm_d)

        # p-state warmup: the PE ramps to full clock only after 3us of
        # continuous execution; spin it on throwaway matmuls while the first
        # input DMAs are in flight so the projections run at full rate
        spin_src = persist.tile([128, 256], BF16, tag="spin_src")
        nc.vector.memset(spin_src, 1.0)
        spin_ps = psum1.tile([128, 512], F32, tag="ps_cv", name="spin_ps")
        for _ in range(4):
            nc.tensor.matmul(spin_ps[0:1, :256], ones_s, spin_src,
                             start=True, stop=True)
        spin_sink = small.tile([1, 16], F32, tag="spin_sink")
        nc.vector.tensor_copy(spin_sink, spin_ps[0:1, :16])  # ring consumer

        # ---- projections: x2 [ch, N], x1T [j, i], x3 [ch, KW] (all bf16) --
        x2_s = persist.tile([128, N], BF16, tag="x2")
        x1t_s = persist.tile([128, 32, Ch], BF16, tag="x1t")
        x3_s = persist.tile([128, KW], BF16, tag="x3")

        def jc_slices(jc, lo, hi):
            """moving-operand slices [lo:hi) of in0 quarter jc (jc 0 is split
            into a 128-col head so the first matmul follows a small DMA)."""
            if jc > 0:
                return [(in0q[jc], lo, hi, lo)]
            out = []
            if lo < 128:
                out.append((in0q0a, lo, min(hi, 128), lo))
            if hi > 128:
                out.append((in0q0b, max(lo, 128) - 128, hi - 128,
                            max(lo, 128)))
            return out

        for jc in range(8):
            # x2 chunk: one accumulation group in one PSUM bank
            ps2 = psum.tile([128, 2, 512], F32, tag="ps_S")
            mms = [(t, a, b, o, cc) for cc in range(2)
                   for (t, a, b, o) in jc_slices(jc, 0, 512)]
            for i, (t, a, b, o, cc) in enumerate(mms):
                nc.tensor.matmul(ps2[:, 0, o:o + b - a], wk_s[:, cc, :],
                                 t[:, cc, a:b],
                                 start=(i == 0), stop=(i == len(mms) - 1))
            nc.vector.tensor_copy(x2_s[:, jc * 512:(jc + 1) * 512], ps2[:, 0, :])
            # x1t: all 4 j-subchunks in ONE group/bank, evicted in one ACT copy
            ps1 = psum1.tile([128, 512], F32, tag="ps_acc")
            mms = [(js, t, a, b, cc) for js in range(4) for cc in range(2)
                   for (t, a, b, _) in jc_slices(jc, js * 128, (js + 1) * 128)]
            for i, (js, t, a, b, cc) in enumerate(mms):
                nc.tensor.matmul(
                    ps1[:, js * 128:js * 128 + Ch],
                    t[:, cc, a:b], wq_s[:, cc, :],
                    start=(i == 0), stop=(i == len(mms) - 1))
            # x1t eviction: DVE while ACT's queue is still dispatching the
            # early input DMAs (~6.3us), ACT once it has drained
            if jc < 5:
                nc.vector.tensor_copy(
                    x1t_s[:, jc * 4:(jc + 1) * 4, :],
                    ps1.rearrange("p (a c) -> p a c", c=Ch))
            else:
                nc.scalar.activation(
                    x1t_s[:, jc * 4:(jc + 1) * 4, :],
                    ps1.rearrange("p (a c) -> p a c", c=Ch), AF.Copy)

        for k0, ksz in K_TILES:
            iq, off = (0, k0) if k0 < 1024 else (1, k0 - 1024)
            ps3 = psum.tile([128, 2, 512], F32, tag="ps_S")
            for cc in range(2):
                nc.tensor.matmul(ps3[:, 0, :ksz], wv_s[:, cc, :],
                                 in1q[iq][:, cc, off:off + ksz],
                                 start=(cc == 0), stop=(cc == 1))
            # x3 = psum + b_v : folds the v-bias into the affinity logits
            nc.vector.tensor_scalar_add(x3_s[:, k0:k0 + ksz], ps3[:, 0, :ksz],
                                        bv_s)

        # ---- conv buffers (fp8, padded 66-wide rows with zero ring) ----
        # ybuf and c0 are split into top (rows 0..18) / bottom (rows 16..35)
        # tiles: conv row-tiles 0-1 read only top, 2-4 only bottom, so each
        # conv stage starts as soon as its first producer rows land instead
        # of waiting for the whole buffer (Tile deps are whole-tile).
        TOPR, BOTR0 = 19, 16  # top rows [0,19), bottom rows [16,36)
        convbuf = ctx.enter_context(tc.tile_pool(name="convbuf", bufs=1))
        ybufT = convbuf.tile([128, 2, TOPR, 66], F8, tag="ybufT")
        ybufB = convbuf.tile([128, 2, 36 - BOTR0, 66], F8, tag="ybufB")
        in0h_s = convbuf.tile([128, 2, 36, 66], F8, tag="in0h")
        in0l_s = convbuf.tile([128, 2, 36, 66], F8, tag="in0l")
        c0hT = convbuf.tile([128, 2, TOPR, 66], F8, tag="c0hT")
        c0hB = convbuf.tile([128, 2, 36 - BOTR0, 66], F8, tag="c0hB")
        c0lT = convbuf.tile([128, 2, TOPR, 66], F8, tag="c0lT")
        c0lB = convbuf.tile([128, 2, 36 - BOTR0, 66], F8, tag="c0lB")
        c0f = convbuf.tile([128, 2, ROWS, W], BF16, tag="c0f")
        for tl in (in0h_s, in0l_s):
            # zero pad rows on the otherwise-idle Pool engine (write-only
            # memset; reading uninitialized SBUF can produce NaNs); side
            # columns arrive zero-padded via the DMA
            nc.gpsimd.memset(tl[:, :, 0, :], 0.0)
            nc.gpsimd.memset(tl[:, :, 35, :], 0.0)
        for tl in (ybufT, c0hT, c0lT):
            nc.gpsimd.memset(tl[:, :, 0, :], 0.0)
            nc.gpsimd.memset(tl[:, :, 1:, 0:1], 0.0)
            nc.gpsimd.memset(tl[:, :, 1:, 65:66], 0.0)
        for tl in (ybufB, c0hB, c0lB):
            nc.gpsimd.memset(tl[:, :, 36 - BOTR0 - 1, :], 0.0)
            nc.gpsimd.memset(tl[:, :, :36 - BOTR0 - 1, 0:1], 0.0)
            nc.gpsimd.memset(tl[:, :, :36 - BOTR0 - 1, 65:66], 0.0)

        def split_rows(r0, nrows):
            """Global padded rows [r0, r0+nrows) -> [(tile, local_r0, n,
            src_off)] covering top and bottom tiles (overlap rows go to
            both)."""
            parts = []
            t_hi = min(r0 + nrows, TOPR)
            if r0 < TOPR:
                parts.append(("T", r0, t_hi - r0, 0))
            b_lo = max(r0, BOTR0)
            if r0 + nrows > BOTR0:
                parts.append(("B", b_lo - BOTR0, r0 + nrows - b_lo,
                              b_lo - r0))
            return parts

        def rd(r0):
            """conv read window [r0-1, r0-1+nr+2) maps to exactly one tile"""
            return ("T", r0) if r0 < 16 else ("B", r0 - BOTR0)
        # in0 conv window ships as fp8 hi/lo straight into the padded tiles
        # (host pre-pads the 66-col side ring so the DMA stays 3-dim)
        nc.sync.dma_start(
            out=in0h_s[:, :, 1:35, :],
            in_=in0h_d.rearrange("(a p) n -> p a n", a=2))
        nc.sync.dma_start(
            out=in0l_s[:, :, 1:35, :],
            in_=in0l_d.rearrange("(a p) n -> p a n", a=2))

        # ---- conv0 weights (early: the in0-half partial sums run inside the
        # attention phase to fill PE slack while ACT grinds the exps) ----
        w0all_s = persist.tile([128, 54, C], F8, tag="w0all")
        nc.sync.dma_start(
            out=w0all_s,
            in_=w0all_d.rearrange("s t (a p) o -> p (s t a) o", a=2))
        w0x_s = w0all_s[:, 0:18]
        w0inh_s = w0all_s[:, 18:36]
        w0inl_s = w0all_s[:, 36:54]
        inpart = [persist.tile([128, 512], BF16, tag=f"inpart{g}",
                               name=f"inpart{g}") for g in range(10)]
        terms_in0 = [(w0inh_s, in0h_s), (w0inh_s, in0l_s), (w0inl_s, in0h_s)]

        def emit_in0_partial(g, tag="ps_cv"):
            """27 DoubleRow passes of conv0's in0-half for group g=(rt,oc),
            evicted to SBUF bf16 for later re-injection."""
            (r0, nr), oc = ROW_TILES[g // 2], g % 2
            ps = psum1.tile([128, 512], F32, tag=tag, name=f"cv{g}")
            pcv = ps[:, :nr * W].rearrange("p (r w) -> p r w", w=W)
            i_mm, n_mm = 0, 27
            for w_s, x_s in terms_in0:
                for t9 in range(9):
                    dh, dw = divmod(t9, 3)
                    nc.tensor.matmul(
                        pcv,
                        w_s[:, t9 * 2:t9 * 2 + 2, oc * 128:(oc + 1) * 128],
                        x_s[:, :, r0 + dh - 1:r0 + dh - 1 + nr, dw:dw + W],
                        start=(i_mm == 0), stop=(i_mm == n_mm - 1),
                        perf_mode=DR)
                    i_mm += 1
            nc.vector.tensor_copy(inpart[g][:, :nr * W], ps[:, :nr * W])

        # ---- attention: S = x2^T x3, exp, x0 = x1 @ exp, sums, normalize ----
        attn = ctx.enter_context(tc.tile_pool(name="attn", bufs=4))
        attn2 = ctx.enter_context(tc.tile_pool(name="attn2", bufs=2))
        dram = ctx.enter_context(tc.tile_pool(name="dram", bufs=5, space="DRAM"))
        # per-k-tile x0n tiles: whole-tile deps would otherwise make each
        # out-projection wait for the LAST k-tile's normalization
        x0n_t = [persist.tile([128, 512], BF16, tag=f"x0n{kt}",
                              name=f"x0n{kt}") for kt in range(len(K_TILES))]
        for kt, (k0, ksz) in enumerate(K_TILES):
            # four quarter-tiles under one bufs=4 tag: stage-2 consumes a
            # quarter while later quarters' exps still run, and the next
            # k-tile's exps begin as soon as a quarter is drained
            expS_h = [attn.tile([128, 8, 512], BF16, tag="expS",
                                name=f"expS{k0}_{h}") for h in range(4)]
            # ssum shares the ps_cv ring with the conv0 in0-half partials:
            # both have fast consumers so the rotation never stalls the PE
            ssum_t = psum1.tile([128, 512], F32, tag="ps_cv",
                                name=f"ssum{k0}")
            for mh in range(16):  # chunk pairs
                sp = psum.tile([128, 2, 512], F32, tag="ps_S")
                for i in range(2):
                    m = 2 * mh + i
                    nc.tensor.matmul(
                        sp[:, i, :ksz],
                        x2_s[:, m * 128:(m + 1) * 128],
                        x3_s[:, k0:k0 + ksz],
                        start=True, stop=True)
                eh = expS_h[mh // 4]
                nc.scalar.activation(
                    eh[:, (2 * mh) % 8:(2 * mh) % 8 + 2, :ksz],
                    sp[:, :, :ksz], AF.Exp)
            # 5-level bf16 pre-sum tree on DVE collapses the softmax
            # column-sum to ONE ones-matmul pass (sum error ~0.3%, only
            # scales the normalization)
            octs = attn2.tile([128, 4, 512], BF16, tag="oct")
            for h in range(4):
                pair = attn.tile([128, 4, 512], BF16, tag="pair",
                                 name=f"pair{k0}_{h}")
                for i in range(4):
                    nc.vector.tensor_add(pair[:, i, :ksz],
                                         expS_h[h][:, 2 * i, :ksz],
                                         expS_h[h][:, 2 * i + 1, :ksz])
                quad = attn.tile([128, 2, 512], BF16, tag="quad",
                                 name=f"quad{k0}_{h}")
                for i in range(2):
                    nc.vector.tensor_add(quad[:, i, :ksz],
                                         pair[:, 2 * i, :ksz],
                                         pair[:, 2 * i + 1, :ksz])
                nc.vector.tensor_add(octs[:, h, :ksz], quad[:, 0, :ksz],
                                     quad[:, 1, :ksz])
            hexs = attn2.tile([128, 2, 512], BF16, tag="hex")
            for i in range(2):
                nc.vector.tensor_add(hexs[:, i, :ksz], octs[:, 2 * i, :ksz],
                                     octs[:, 2 * i + 1, :ksz])
            top = attn2.tile([128, 512], BF16, tag="top")
            nc.vector.tensor_add(top[:, :ksz], hexs[:, 0, :ksz],
                                 hexs[:, 1, :ksz])
            if kt == 0:
                # fill the one-time exp-pipeline-fill bubble (the first x0
                # matmul needs a full expS quarter = 8 exps of ACT lead time)
                emit_in0_partial(3, tag="ps_acc")
            x0p = psum1.tile([128, 512], F32, tag="ps_acc")
            ssum = ssum_t[0:1, :]
            for m in range(32):
                eSm = expS_h[m // 8][:, m % 8, :ksz]
                nc.tensor.matmul(x0p[:, :ksz], x1t_s[:, m, :], eSm,
                                 start=(m == 0), stop=(m == 31))
            # one conv0 in0-half partial fills the tree's DVE latency and
            # balances attention PE load vs the exps; most siblings run in
            # the ACT-bound out-projection phase instead
            emit_in0_partial(2 * kt)
            if kt == 4:
                # and one more bridges the last k-tile's serial
                # tree->ssum->recip->broadcast tail
                emit_in0_partial(1, tag="ps_acc")
            nc.tensor.matmul(ssum[:, :ksz], ones_s, top[:, :ksz],
                             start=True, stop=True)
            sinv = small.tile([1, 512], BF16, tag="sinv")
            with nc.allow_low_precision(
                    reason="bf16 1/colsum only scales the softmax "
                           "normalization; 1.2e-2 in the numpy bit model"):
                nc.vector.reciprocal(sinv[:, :ksz], ssum[:, :ksz])
            # broadcast 1/colsum to all partitions with a 1-row bf16 matmul
            # (a DRAM roundtrip would put ~3us of DMA latency on this chain)
            bcast = psum1.tile([128, 512], F32, tag="ps_cv",
                               name=f"sinv_bcast{kt}")
            nc.tensor.matmul(bcast[:, :ksz], ones_row, sinv[:, :ksz],
                             start=True, stop=True)
            sinvb = small.tile([128, 512], F32, tag="sinvb")
            nc.scalar.activation(sinvb[:, :ksz], bcast[:, :ksz], AF.Copy)
            nc.vector.tensor_mul(x0n_t[kt][:, :ksz], x0p[:, :ksz],
                                 sinvb[:, :ksz])

        # ---- out-projection + bn0 + gelu -> x0' (fp8) into ybuf T/B.
        # Per-k-tile x0n tiles + split ybuf let conv0 start after the first
        # three gelu writes instead of all ten. ----
        for kt, (k0, ksz) in enumerate(K_TILES):
            nr = ksz // W
            for oc in range(2):
                po = psum1.tile([128, 512], F32, name=f"po{kt}_{oc}",
                                tag="ps_acc" if oc == 0 else "ps_cv")
                nc.tensor.matmul(po[:, :ksz],
                                 wo_s[:, oc * 128:(oc + 1) * 128],
                                 x0n_t[kt][:, :ksz],
                                 start=True, stop=True)
                pv = po[:, :ksz].rearrange("p (r w) -> p r w", w=W)
                for which, lr0, n, src in split_rows(1 + kt * 8, nr):
                    dst = ybufT if which == "T" else ybufB
                    nc.scalar.activation(
                        dst[:, oc, lr0:lr0 + n, 1:65], pv[:, src:src + n],
                        AF.Gelu, bias=bo_s[:, oc:oc + 1],
                        scale=ao_s[:, oc:oc + 1])
            # leftover in0-half partials: this phase is ACT-gelu-bound
            # (g1 and g3 were pulled into the attention phase's bubbles)
            if kt >= 2:
                emit_in0_partial(2 * kt + 1, tag="ps_acc")

        # ---- conv1 weights (loaded during attention; fp8 hi/lo) ----
        w1all_s = persist.tile([128, 36, C], F8, tag="w1all")
        nc.sync.dma_start(
            out=w1all_s,
            in_=w1all_d.rearrange("s t (a p) o -> p (s t a) o", a=2))
        w1h_s = w1all_s[:, 0:18]
        w1l_s = w1all_s[:, 18:36]

        # ---- conv0: x0'-half naive fp8 DoubleRow on top of the re-injected
        # in0-half partial (identity matmul opens the accumulation) ----
        for ri, (r0, nr) in enumerate(ROW_TILES):
            for oc in range(2):
                pc = psum1.tile([128, 512], F32, name=f"c0ps{ri}_{oc}",
                                tag="ps_acc" if oc == 0 else "ps_cv")
                pcv = pc[:, :nr * W].rearrange("p (r w) -> p r w", w=W)
                which, lr0 = rd(r0)
                ysrc = ybufT if which == "T" else ybufB
                nc.tensor.matmul(pc[:, :nr * W], id_s,
                                 inpart[ri * 2 + oc][:, :nr * W],
                                 start=True, stop=False)
                for t9 in range(9):
                    dh, dw = divmod(t9, 3)
                    nc.tensor.matmul(
                        pcv,
                        w0x_s[:, t9 * 2:t9 * 2 + 2, oc * 128:(oc + 1) * 128],
                        ysrc[:, :, lr0 + dh - 1:lr0 + dh - 1 + nr, dw:dw + W],
                        start=False, stop=(t9 == 8),
                        perf_mode=DR)
                nc.scalar.activation(
                    c0f[:, oc, r0 - 1:r0 - 1 + nr, :], pcv,
                    AF.Gelu, bias=b0_s[:, oc:oc + 1], scale=a0_s[:, oc:oc + 1])
                # hi/lo split of c0 for conv1's 3-term product (DVE), into
                # the top/bottom tiles (boundary rows land in both)
                for w2, lr2, n2, src2 in split_rows(r0, nr):
                    ch = c0hT if w2 == "T" else c0hB
                    cl = c0lT if w2 == "T" else c0lB
                    s2 = r0 - 1 + src2
                    nc.vector.tensor_copy(ch[:, oc, lr2:lr2 + n2, 1:65],
                                          c0f[:, oc, s2:s2 + n2, :])
                    nc.vector.tensor_sub(cl[:, oc, lr2:lr2 + n2, 1:65],
                                         c0f[:, oc, s2:s2 + n2, :],
                                         ch[:, oc, lr2:lr2 + n2, 1:65])

        # ---- conv1: 256 -> 256, 3-term DoubleRow fp8, bn + gelu,
        #      + x0' residual, row-max; per-row-tile output DMA so only the
        #      small last tile sits on the kernel tail ----
        for ri, (r0, nr) in enumerate(ROW_TILES):
            for oc in range(2):
                pc = psum1.tile([128, 512], F32, name=f"c1ps{ri}_{oc}",
                                tag="ps_acc" if oc == 0 else "ps_cv")
                pcv = pc[:, :nr * W].rearrange("p (r w) -> p r w", w=W)
                which, lr0 = rd(r0)
                ch = c0hT if which == "T" else c0hB
                cl = c0lT if which == "T" else c0lB
                terms1 = [(w1h_s, ch), (w1h_s, cl), (w1l_s, ch)]
                i_mm, n_mm = 0, 9 * len(terms1)
                for w_s, x_s in terms1:
                    for t9 in range(9):
                        dh, dw = divmod(t9, 3)
                        nc.tensor.matmul(
                            pcv,
                            w_s[:, t9 * 2:t9 * 2 + 2, oc * 128:(oc + 1) * 128],
                            x_s[:, :, lr0 + dh - 1:lr0 + dh - 1 + nr,
                                dw:dw + W],
                            start=(i_mm == 0), stop=(i_mm == n_mm - 1),
                            perf_mode=DR)
                        i_mm += 1
                tmp = small.tile([128, 512], F32, tag="scratch")
                nc.scalar.activation(tmp[:, :nr * W], pc[:, :nr * W], AF.Gelu,
                                     bias=b1_s[:, oc:oc + 1],
                                     scale=a1_s[:, oc:oc + 1])
                yres = ybufT if r0 + nr <= TOPR else ybufB
                yr0 = r0 if r0 + nr <= TOPR else r0 - BOTR0
                # final row tile finishes on the idle Pool engine, skipping
                # the backlogged DVE queue on the kernel tail
                veng = nc.gpsimd if ri == len(ROW_TILES) - 1 else nc.vector
                res = small.tile([128, 512], F32, tag="scratch")
                veng.tensor_add(
                    res[:, :nr * W].rearrange("p (r w) -> p r w", w=W),
                    tmp[:, :nr * W].rearrange("p (r w) -> p r w", w=W),
                    yres[:, oc, yr0:yr0 + nr, 1:65])
                outr = small.tile([128, 8], F32, tag="outr")
                nc.vector.reduce_max(
                    outr[:, :nr],
                    res[:, :nr * W].rearrange("p (r w) -> p r w", w=W),
                    axis=AX.X)
                # SP queue only: ACT dispatches would delay the last gelus
                nc.sync.dma_start(
                    out=out[oc * 128:(oc + 1) * 128, r0 - 1:r0 - 1 + nr],
                    in_=outr[:, :nr])

    nc.compile()
    return nc


def _prep_maps(inputs):
    """Host-side input prep: slicing, transposes, BN folding, fp8 splits."""
    f = np.float32
    in0 = np.ascontiguousarray(np.asarray(inputs["inputs_0"], f).reshape(B, C, N))
    in1 = np.ascontiguousarray(np.asarray(inputs["inputs_1"], f).reshape(B, C, N))
    g = {k: np.asarray(v, f) for k, v in inputs.items()}

    def fold(gm, bt, m, v, conv_b):
        a = (gm / np.sqrt(v + EPS)).astype(f)
        return a, (bt - m * a + a * conv_b).astype(f)

    a_bn, b_bn = fold(g["bn0_g"], g["bn0_b"], g["bn0_m"], g["bn0_v"],
                      g["b_o"] + g["w_o"] @ g["b_q"])
    a0, b0 = fold(g["cb_bn0_g"], g["cb_bn0_b"], g["cb_bn0_m"], g["cb_bn0_v"],
                  g["cb_b0"])
    a1, b1 = fold(g["cb_bn1_g"], g["cb_bn1_b"], g["cb_bn1_m"], g["cb_bn1_v"],
                  g["cb_b1"])

    def wsplit(w):
        wh = w.astype(F8NP)
        wl = (w - wh.astype(f)).astype(F8NP)
        return wh, wl

    # conv weights as (tap, ci, o); x0-half naive fp8, in0-half + w1 hi/lo
    w0t = np.ascontiguousarray(
        g["cb_w0"].transpose(2, 3, 1, 0).reshape(9, 2 * C, C))
    w1t = np.ascontiguousarray(
        g["cb_w1"].transpose(2, 3, 1, 0).reshape(9, C, C))
    w0inh, w0inl = wsplit(w0t[:, C:, :])
    w1h, w1l = wsplit(w1t)

    shared = {
        "wq_t": np.ascontiguousarray(g["w_q"].T).astype(BF16NP),
        "wk_t": np.ascontiguousarray(g["w_k"].T).astype(BF16NP),
        "wv_t": np.ascontiguousarray(g["w_v"].T).astype(BF16NP),
        "wo_t": np.ascontiguousarray(g["w_o"].T).astype(BF16NP),
        "bv": np.ascontiguousarray(g["b_v"].reshape(Ch, 1)),
        "bias6": np.ascontiguousarray(
            np.stack([a_bn, b_bn, a0, b0, a1, b1]).reshape(6, C, 1)),
        "w0all": np.ascontiguousarray(
            np.stack([w0t[:, :C, :].astype(F8NP), w0inh, w0inl])),
        "w1all": np.ascontiguousarray(np.stack([w1h, w1l])),
        "idm": np.eye(128, dtype=BF16NP),
    }
    maps = []
    for b in range(B):
        in0b16 = in0[b].astype(BF16NP)
        for half in range(2):
            w0r = 0 if half == 0 else 30
            sl = slice(w0r * W, (w0r + ROWS) * W)
            in0w_f32 = in0[b][:, sl].reshape(C, ROWS, W)
            in0h = np.zeros((C, ROWS, 66), F8NP)
            in0l = np.zeros((C, ROWS, 66), F8NP)
            in0h[:, :, 1:65] = in0w_f32.astype(F8NP)
            in0l[:, :, 1:65] = (
                in0w_f32 - in0h[:, :, 1:65].astype(f)).astype(F8NP)
            maps.append({
                "in0b": in0b16,
                "in0h": in0h.reshape(C, ROWS * 66),
                "in0l": in0l.reshape(C, ROWS * 66),
                "in1b": np.ascontiguousarray(in1[b][:, sl]).astype(BF16NP),
                **shared,
            })
    return maps


def kernel(**inputs):
    if "nc" not in _CACHED:
        _CACHED["nc"] = build_program()
    nc = _CACHED["nc"]
    maps = _prep_maps(inputs)
    res = run_bass_kernel_spmd(nc, maps, core_ids=list(range(8)))
    out = np.zeros((B, C), np.float32)
    for b in range(B):
        top = res.results[2 * b]["out"][:, 0:32].max(axis=1)
        bot = res.results[2 * b + 1]["out"][:, 2:34].max(axis=1)
        out[b] = np.maximum(out[b], np.maximum(top, bot))
    return out


# revision 70
# speedup vs baseline: 1.0152x; 1.0152x over previous
"""Trainium2 Bass kernel for nn_CFAConv (cross-feature attention + conv block).

Self-contained: takes full unsharded inputs, shards (batch, image-half) across
8 NeuronCores, runs one SPMD Bass/Tile NEFF, and combines partial results on
the host.

Math (validated against the jax reference in numpy):
  x1 = w_q@in0 + b_q ; x2 = w_k@in0 + b_k ; x3 = w_v@in1 + b_v  (1x1 convs)
  aff = softmax_j(x2^T x3) ; x0 = x1 @ aff
  x0' = gelu(bn0(w_o@x0 + b_o))
  y = gelu(bn(conv3x3(concat(x0', in0)))) ; y = gelu(bn(conv3x3(y)))
  out = max_spatial(y + x0')
On-device simplifications:
  - softmax over j is invariant to per-column shifts => b_k drops entirely
  - x2^T(x3 + b_v) = x2^T x3 + (x2^T b_v) 1^T    => fold b_v into x3
  - (x1 + b_q 1^T) @ aff = x1@aff + b_q 1^T (aff columns sum to 1)
    => fold w_o@b_q into the out-projection bias (host-side)
  - eval-mode BN folds to per-channel scale/bias, fused into the gelu ACT op
  - softmax normalization deferred past the x1@exp(S) matmul (divide x0 by
    column sums); sums via a 5-level bf16 DVE pre-sum tree + one ones-matmul
  - no max-subtraction in softmax: |S| <= ~60 here; exp fits fp32 (max ~e88)
Precision: bf16 operands with fp32 PSUM accumulation for the attention path;
the two 3x3 convs run in fp8e4m3 with DoubleRow perf mode (2 contraction
tiles per pass at 0.5 cycles/row):
  - conv0 x0'-half: weights + acts naive fp8 (x0' is small vs in0 => cheap)
  - conv0 in0-half: weights hi+lo fp8 split, in0 hi+lo fp8 split (host-side),
    3-term product (Wh Xh + Wh Xl + Wl Xh)
  - conv1: weights hi+lo (host), c0 hi+lo split on DVE, 3-term
  (numpy bit-model: 1.3e-2 final rel err vs the 2e-2 budget)
Sharding: 8 cores = (4 batches) x (top/bottom image half). Each core computes
a 34-row window (32 owned + halo) so the two 3x3 convs need no communication;
per-row maxes [256, 34] go to the host which slices owned rows and reduces.
"""

from contextlib import ExitStack

import ml_dtypes
import numpy as np

import concourse.bass as bass
import concourse.tile as tile
from concourse import bacc, mybir
from concourse.bass_utils import run_bass_kernel_spmd

B, C, H, W = 4, 256, 64, 64
Ch = C // 2          # 128
N = H * W            # 4096
ROWS = 34            # per-core row window (32 owned + 2 halo)
KW = ROWS * W        # 2176 window positions
EPS = 1e-5

F32 = mybir.dt.float32
BF16 = mybir.dt.bfloat16
F8 = mybir.dt.float8e4
AF = mybir.ActivationFunctionType
AX = mybir.AxisListType
DR = mybir.MatmulPerfMode.DoubleRow
BF16NP = ml_dtypes.bfloat16
F8NP = ml_dtypes.float8_e4m3

# attention k-tiles over the 2176-column window
K_TILES = [(0, 512), (512, 512), (1024, 512), (1536, 512), (2048, 128)]
# conv output row-tiles (local rows 1..34 of the 36-row padded buffer)
ROW_TILES = [(1, 8), (9, 8), (17, 8), (25, 8), (33, 2)]

_CACHED = {}


def build_program():
    nc = bacc.Bacc("TRN2", target_bir_lowering=False, debug=False)

    def din(name, shape, dt=F32):
        return nc.dram_tensor(name, shape, dt, kind="ExternalInput").ap()

    in0b_d = din("in0b", [C, N], BF16)
    in1b_d = din("in1b", [C, KW], BF16)
    # in0 conv window, fp8 hi/lo, pre-padded to 66 cols (zero side columns)
    in0h_d = din("in0h", [C, ROWS * 66], F8)
    in0l_d = din("in0l", [C, ROWS * 66], F8)
    wq_t = din("wq_t", [C, Ch], BF16)     # (c, i)
    wk_t = din("wk_t", [C, Ch], BF16)
    wv_t = din("wv_t", [C, Ch], BF16)
    wo_t = din("wo_t", [Ch, C], BF16)     # (i, o)
    bv = din("bv", [Ch, 1])
    bias6_d = din("bias6", [6, C, 1])     # ao, bo, a0, b0, a1, b1
    # conv0 weights: [x0-half naive, in0-half hi, in0-half lo] (tap, ci, o)
    w0all_d = din("w0all", [3, 9, C, C], F8)
    w1all_d = din("w1all", [2, 9, C, C], F8)  # [hi, lo]
    idm_d = din("idm", [128, 128], BF16)
    out = nc.dram_tensor("out", [C, ROWS], F32, kind="ExternalOutput").ap()

    with tile.TileContext(nc) as tc, ExitStack() as ctx:
        persist = ctx.enter_context(tc.tile_pool(name="persist", bufs=1))
        psum = ctx.enter_context(tc.tile_pool(name="psum", bufs=2, space="PSUM"))
        psum1 = ctx.enter_context(tc.tile_pool(name="psum1", bufs=2, space="PSUM"))
        small = ctx.enter_context(tc.tile_pool(name="small", bufs=3))

        # ---- inputs: bf16 quarters of in0 (one DMA each: per-slice deps
        # because Tile dependencies are whole-tile). Weights + a small first
        # slice of in0 go first so the first matmul starts ASAP. ----
        # DMA issue costs ~1.26us/queue: strictly alternate the projection
        # inputs across the SP and ACT HWDGE queues in consumption order
        in0ap = in0b_d.rearrange("(a p) n -> p a n", a=2)
        in0q0a = persist.tile([128, 2, 128], BF16, tag="in0q0a")
        nc.scalar.dma_start(out=in0q0a, in_=in0ap[:, :, 0:128])
        wk_s = persist.tile([128, 2, Ch], BF16, tag="wk")
        nc.sync.dma_start(out=wk_s, in_=wk_t.rearrange("(a p) n -> p a n", a=2))
        in0q0b = persist.tile([128, 2, 384], BF16, tag="in0q0b")
        nc.scalar.dma_start(out=in0q0b, in_=in0ap[:, :, 128:512])
        wq_s = persist.tile([128, 2, Ch], BF16, tag="wq")
        nc.sync.dma_start(out=wq_s, in_=wq_t.rearrange("(a p) n -> p a n", a=2))
        in0q = [None] + [persist.tile([128, 2, 512], BF16, tag=f"in0q{q}",
                                      name=f"in0q{q}") for q in range(1, 8)]
        in1q = [persist.tile([128, 2, 1152], BF16, tag=f"in1q{q}",
                             name=f"in1q{q}") for q in range(2)]
        for q in range(1, 8):
            eng = nc.scalar if q % 2 == 0 else nc.sync
            eng.dma_start(out=in0q[q], in_=in0ap[:, :, q * 512:(q + 1) * 512])
        nc.sync.dma_start(
            out=in1q[0][:, :, :1024],
            in_=in1b_d.rearrange("(a p) n -> p a n", a=2)[:, :, 0:1024])
        nc.scalar.dma_start(
            out=in1q[1],
            in_=in1b_d.rearrange("(a p) n -> p a n", a=2)[:, :, 1024:KW])
        wv_s = persist.tile([128, 2, Ch], BF16, tag="wv")
        nc.scalar.dma_start(out=wv_s,
                            in_=wv_t.rearrange("(a p) n -> p a n", a=2))
        bv_s = persist.tile([128, 1], F32, tag="bv")
        nc.scalar.dma_start(out=bv_s, in_=bv)
        wo_s = persist.tile([128, C], BF16, tag="wo")
        nc.sync.dma_start(out=wo_s, in_=wo_t)
        bias_s = persist.tile([128, 12], F32, tag="bias6")
        nc.sync.dma_start(out=bias_s,
                          in_=bias6_d.rearrange("t (a p) o -> p (t a o)", a=2))
        ao_s, bo_s = bias_s[:, 0:2], bias_s[:, 2:4]
        a0_s, b0_s = bias_s[:, 4:6], bias_s[:, 6:8]
        a1_s, b1_s = bias_s[:, 8:10], bias_s[:, 10:12]
        ones_s = persist.tile([128, 1], BF16, tag="ones")
        nc.vector.memset(ones_s, 1.0)
        ones_row = persist.tile([1, 128], BF16, tag="ones_row")
        nc.vector.memset(ones_row, 1.0)
        id_s = persist.tile([128, 128], BF16, tag="idm")
        nc.sync.dma_start(out=id_s, in_=idm_d)

        # p-state warmup: the PE ramps to full clock only after 3us of
        # continuous execution; spin it on throwaway matmuls while the first
        # input DMAs are in flight so the projections run at full rate
        spin_src = persist.tile([128, 256], BF16, tag="spin_src")
        nc.vector.memset(spin_src, 1.0)
        spin_ps = psum1.tile([128, 512], F32, tag="ps_cv", name="spin_ps")
        for _ in range(7):
            nc.tensor.matmul(spin_ps[0:1, :256], ones_s, spin_src,
                             start=True, stop=True)
        spin_sink = small.tile([1, 16], F32, tag="spin_sink")
        nc.vector.tensor_copy(spin_sink, spin_ps[0:1, :16])  # ring consumer

        # ---- projections: x2 [ch, N], x1T [j, i], x3 [ch, KW] (all bf16) --
        x2_s = persist.tile([128, N], BF16, tag="x2")
        x1t_s = persist.tile([128, 32, Ch], BF16, tag="x1t")
        x3_s = persist.tile([128, KW], BF16, tag="x3")

        def jc_slices(jc, lo, hi):
            """moving-operand slices [lo:hi) of in0 quarter jc (jc 0 is split
            into a 128-col head so the first matmul follows a small DMA)."""
            if jc > 0:
                return [(in0q[jc], lo, hi, lo)]
            out = []
            if lo < 128:
                out.append((in0q0a, lo, min(hi, 128), lo))
            if hi > 128:
                out.append((in0q0b, max(lo, 128) - 128, hi - 128,
                            max(lo, 128)))
            return out

        for jc in range(8):
            # x2 chunk: one accumulation group in one PSUM bank
            ps2 = psum.tile([128, 2, 512], F32, tag="ps_S")
            mms = [(t, a, b, o, cc) for cc in range(2)
                   for (t, a, b, o) in jc_slices(jc, 0, 512)]
            for i, (t, a, b, o, cc) in enumerate(mms):
                nc.tensor.matmul(ps2[:, 0, o:o + b - a], wk_s[:, cc, :],
                                 t[:, cc, a:b],
                                 start=(i == 0), stop=(i == len(mms) - 1))
            nc.vector.tensor_copy(x2_s[:, jc * 512:(jc + 1) * 512], ps2[:, 0, :])
            # x1t: all 4 j-subchunks in ONE group/bank, evicted in one ACT copy
            ps1 = psum1.tile([128, 512], F32, tag="ps_acc")
            mms = [(js, t, a, b, cc) for js in range(4) for cc in range(2)
                   for (t, a, b, _) in jc_slices(jc, js * 128, (js + 1) * 128)]
            for i, (js, t, a, b, cc) in enumerate(mms):
                nc.tensor.matmul(
                    ps1[:, js * 128:js * 128 + Ch],
                    t[:, cc, a:b], wq_s[:, cc, :],
                    start=(i == 0), stop=(i == len(mms) - 1))
            # x1t eviction: DVE while ACT's queue is still dispatching the
            # early input DMAs (~6.3us), ACT once it has drained
            if jc < 5:
                nc.vector.tensor_copy(
                    x1t_s[:, jc * 4:(jc + 1) * 4, :],
                    ps1.rearrange("p (a c) -> p a c", c=Ch))
            else:
                nc.scalar.activation(
                    x1t_s[:, jc * 4:(jc + 1) * 4, :],
                    ps1.rearrange("p (a c) -> p a c", c=Ch), AF.Copy)

        for k0, ksz in K_TILES:
            iq, off = (0, k0) if k0 < 1024 else (1, k0 - 1024)
            ps3 = psum.tile([128, 2, 512], F32, tag="ps_S")
            for cc in range(2):
                nc.tensor.matmul(ps3[:, 0, :ksz], wv_s[:, cc, :],
                                 in1q[iq][:, cc, off:off + ksz],
                                 start=(cc == 0), stop=(cc == 1))
            # x3 = psum + b_v : folds the v-bias into the affinity logits
            nc.vector.tensor_scalar_add(x3_s[:, k0:k0 + ksz], ps3[:, 0, :ksz],
                                        bv_s)

        # ---- conv buffers (fp8, padded 66-wide rows with zero ring) ----
        # ybuf and c0 are split into top (rows 0..18) / bottom (rows 16..35)
        # tiles: conv row-tiles 0-1 read only top, 2-4 only bottom, so each
        # conv stage starts as soon as its first producer rows land instead
        # of waiting for the whole buffer (Tile deps are whole-tile).
        TOPR, BOTR0 = 19, 16  # top rows [0,19), bottom rows [16,36)
        convbuf = ctx.enter_context(tc.tile_pool(name="convbuf", bufs=1))
        ybufT = convbuf.tile([128, 2, TOPR, 66], F8, tag="ybufT")
        ybufB = convbuf.tile([128, 2, 36 - BOTR0, 66], F8, tag="ybufB")
        in0h_s = convbuf.tile([128, 2, 36, 66], F8, tag="in0h")
        in0l_s = convbuf.tile([128, 2, 36, 66], F8, tag="in0l")
        c0hT = convbuf.tile([128, 2, TOPR, 66], F8, tag="c0hT")
        c0hB = convbuf.tile([128, 2, 36 - BOTR0, 66], F8, tag="c0hB")
        c0lT = convbuf.tile([128, 2, TOPR, 66], F8, tag="c0lT")
        c0lB = convbuf.tile([128, 2, 36 - BOTR0, 66], F8, tag="c0lB")
        c0f = convbuf.tile([128, 2, ROWS, W], BF16, tag="c0f")
        for tl in (in0h_s, in0l_s):
            # zero pad rows on the otherwise-idle Pool engine (write-only
            # memset; reading uninitialized SBUF can produce NaNs); side
            # columns arrive zero-padded via the DMA
            nc.gpsimd.memset(tl[:, :, 0, :], 0.0)
            nc.gpsimd.memset(tl[:, :, 35, :], 0.0)
        for tl in (ybufT, c0hT, c0lT):
            nc.gpsimd.memset(tl[:, :, 0, :], 0.0)
            nc.gpsimd.memset(tl[:, :, 1:, 0:1], 0.0)
            nc.gpsimd.memset(tl[:, :, 1:, 65:66], 0.0)
        for tl in (ybufB, c0hB, c0lB):
            nc.gpsimd.memset(tl[:, :, 36 - BOTR0 - 1, :], 0.0)
            nc.gpsimd.memset(tl[:, :, :36 - BOTR0 - 1, 0:1], 0.0)
            nc.gpsimd.memset(tl[:, :, :36 - BOTR0 - 1, 65:66], 0.0)

        def split_rows(r0, nrows):
            """Global padded rows [r0, r0+nrows) -> [(tile, local_r0, n,
            src_off)] covering top and bottom tiles (overlap rows go to
            both)."""
            parts = []
            t_hi = min(r0 + nrows, TOPR)
            if r0 < TOPR:
                parts.append(("T", r0, t_hi - r0, 0))
            b_lo = max(r0, BOTR0)
            if r0 + nrows > BOTR0:
                parts.append(("B", b_lo - BOTR0, r0 + nrows - b_lo,
                              b_lo - r0))
            return parts

        def rd(r0):
            """conv read window [r0-1, r0-1+nr+2) maps to exactly one tile"""
            return ("T", r0) if r0 < 16 else ("B", r0 - BOTR0)
        # in0 conv window ships as fp8 hi/lo straight into the padded tiles
        # (host pre-pads the 66-col side ring so the DMA stays 3-dim)
        nc.sync.dma_start(
            out=in0h_s[:, :, 1:35, :],
            in_=in0h_d.rearrange("(a p) n -> p a n", a=2))
        nc.sync.dma_start(
            out=in0l_s[:, :, 1:35, :],
            in_=in0l_d.rearrange("(a p) n -> p a n", a=2))

        # ---- conv0 weights (early: the in0-half partial sums run inside the
        # attention phase to fill PE slack while ACT grinds the exps) ----
        w0all_s = persist.tile([128, 54, C], F8, tag="w0all")
        nc.sync.dma_start(
            out=w0all_s,
            in_=w0all_d.rearrange("s t (a p) o -> p (s t a) o", a=2))
        w0x_s = w0all_s[:, 0:18]
        w0inh_s = w0all_s[:, 18:36]
        w0inl_s = w0all_s[:, 36:54]
        inpart = [persist.tile([128, 512], BF16, tag=f"inpart{g}",
                               name=f"inpart{g}") for g in range(10)]
        terms_in0 = [(w0inh_s, in0h_s), (w0inh_s, in0l_s), (w0inl_s, in0h_s)]

        def emit_in0_partial(g, tag="ps_cv"):
            """27 DoubleRow passes of conv0's in0-half for group g=(rt,oc),
            evicted to SBUF bf16 for later re-injection."""
            (r0, nr), oc = ROW_TILES[g // 2], g % 2
            ps = psum1.tile([128, 512], F32, tag=tag, name=f"cv{g}")
            pcv = ps[:, :nr * W].rearrange("p (r w) -> p r w", w=W)
            i_mm, n_mm = 0, 27
            for w_s, x_s in terms_in0:
                for t9 in range(9):
                    dh, dw = divmod(t9, 3)
                    nc.tensor.matmul(
                        pcv,
                        w_s[:, t9 * 2:t9 * 2 + 2, oc * 128:(oc + 1) * 128],
                        x_s[:, :, r0 + dh - 1:r0 + dh - 1 + nr, dw:dw + W],
                        start=(i_mm == 0), stop=(i_mm == n_mm - 1),
                        perf_mode=DR)
                    i_mm += 1
            nc.vector.tensor_copy(inpart[g][:, :nr * W], ps[:, :nr * W])

        # ---- attention: S = x2^T x3, exp, x0 = x1 @ exp, sums, normalize ----
        attn = ctx.enter_context(tc.tile_pool(name="attn", bufs=4))
        attn2 = ctx.enter_context(tc.tile_pool(name="attn2", bufs=2))
        dram = ctx.enter_context(tc.tile_pool(name="dram", bufs=5, space="DRAM"))
        # per-k-tile x0n tiles: whole-tile deps would otherwise make each
        # out-projection wait for the LAST k-tile's normalization
        x0n_t = [persist.tile([128, 512], BF16, tag=f"x0n{kt}",
                              name=f"x0n{kt}") for kt in range(len(K_TILES))]
        for kt, (k0, ksz) in enumerate(K_TILES):
            # four quarter-tiles under one bufs=4 tag: stage-2 consumes a
            # quarter while later quarters' exps still run, and the next
            # k-tile's exps begin as soon as a quarter is drained
            expS_h = [attn.tile([128, 8, 512], BF16, tag="expS",
                                name=f"expS{k0}_{h}") for h in range(4)]
            # ssum shares the ps_cv ring with the conv0 in0-half partials:
            # both have fast consumers so the rotation never stalls the PE
            ssum_t = psum1.tile([128, 512], F32, tag="ps_cv",
                                name=f"ssum{k0}")
            for mh in range(16):  # chunk pairs
                sp = psum.tile([128, 2, 512], F32, tag="ps_S")
                for i in range(2):
                    m = 2 * mh + i
                    nc.tensor.matmul(
                        sp[:, i, :ksz],
                        x2_s[:, m * 128:(m + 1) * 128],
                        x3_s[:, k0:k0 + ksz],
                        start=True, stop=True)
                eh = expS_h[mh // 4]
                nc.scalar.activation(
                    eh[:, (2 * mh) % 8:(2 * mh) % 8 + 2, :ksz],
                    sp[:, :, :ksz], AF.Exp)
            # 5-level bf16 pre-sum tree on DVE collapses the softmax
            # column-sum to ONE ones-matmul pass (sum error ~0.3%, only
            # scales the normalization)
            octs = attn2.tile([128, 4, 512], BF16, tag="oct")
            for h in range(4):
                pair = attn.tile([128, 4, 512], BF16, tag="pair",
                                 name=f"pair{k0}_{h}")
                for i in range(4):
                    nc.vector.tensor_add(pair[:, i, :ksz],
                                         expS_h[h][:, 2 * i, :ksz],
                                         expS_h[h][:, 2 * i + 1, :ksz])
                quad = attn.tile([128, 2, 512], BF16, tag="quad",
                                 name=f"quad{k0}_{h}")
                for i in range(2):
                    nc.vector.tensor_add(quad[:, i, :ksz],
                                         pair[:, 2 * i, :ksz],
                                         pair[:, 2 * i + 1, :ksz])
                nc.vector.tensor_add(octs[:, h, :ksz], quad[:, 0, :ksz],
                                     quad[:, 1, :ksz])
            hexs = attn2.tile([128, 2, 512], BF16, tag="hex")
            for i in range(2):
                nc.vector.tensor_add(hexs[:, i, :ksz], octs[:, 2 * i, :ksz],
                                     octs[:, 2 * i + 1, :ksz])
            top = attn2.tile([128, 512], BF16, tag="top")
            nc.vector.tensor_add(top[:, :ksz], hexs[:, 0, :ksz],
                                 hexs[:, 1, :ksz])
            x0p = psum1.tile([128, 512], F32, tag="ps_acc")
            ssum = ssum_t[0:1, :]
            for m in range(32):
                eSm = expS_h[m // 8][:, m % 8, :ksz]
                nc.tensor.matmul(x0p[:, :ksz], x1t_s[:, m, :], eSm,
                                 start=(m == 0), stop=(m == 31))
            # one conv0 in0-half partial fills the tree's DVE latency and
            # balances attention PE load vs the exps; its sibling runs in
            # the ACT-bound out-projection phase instead
            emit_in0_partial(2 * kt)
            nc.tensor.matmul(ssum[:, :ksz], ones_s, top[:, :ksz],
                             start=True, stop=True)
            sinv = small.tile([1, 512], BF16, tag="sinv")
            with nc.allow_low_precision(
                    reason="bf16 1/colsum only scales the softmax "
                           "normalization; 1.2e-2 in the numpy bit model"):
                nc.vector.reciprocal(sinv[:, :ksz], ssum[:, :ksz])
            # broadcast 1/colsum to all partitions with a 1-row bf16 matmul
            # (a DRAM roundtrip would put ~3us of DMA latency on this chain)
            bcast = psum1.tile([128, 512], F32, tag="ps_cv",
                               name=f"sinv_bcast{kt}")
            nc.tensor.matmul(bcast[:, :ksz], ones_row, sinv[:, :ksz],
                             start=True, stop=True)
            sinvb = small.tile([128, 512], F32, tag="sinvb")
            nc.scalar.activation(sinvb[:, :ksz], bcast[:, :ksz], AF.Copy)
            nc.vector.tensor_mul(x0n_t[kt][:, :ksz], x0p[:, :ksz],
                                 sinvb[:, :ksz])

        # ---- out-projection + bn0 + gelu -> x0' (fp8) into ybuf T/B.
        # Per-k-tile x0n tiles + split ybuf let conv0 start after the first
        # three gelu writes instead of all ten. ----
        for kt, (k0, ksz) in enumerate(K_TILES):
            nr = ksz // W
            for oc in range(2):
                po = psum1.tile([128, 512], F32, name=f"po{kt}_{oc}",
                                tag="ps_acc" if oc == 0 else "ps_cv")
                nc.tensor.matmul(po[:, :ksz],
                                 wo_s[:, oc * 128:(oc + 1) * 128],
                                 x0n_t[kt][:, :ksz],
                                 start=True, stop=True)
                pv = po[:, :ksz].rearrange("p (r w) -> p r w", w=W)
                for which, lr0, n, src in split_rows(1 + kt * 8, nr):
                    dst = ybufT if which == "T" else ybufB
                    nc.scalar.activation(
                        dst[:, oc, lr0:lr0 + n, 1:65], pv[:, src:src + n],
                        AF.Gelu, bias=bo_s[:, oc:oc + 1],
                        scale=ao_s[:, oc:oc + 1])
            # leftover in0-half partial: this phase is ACT-gelu-bound
            emit_in0_partial(2 * kt + 1, tag="ps_acc")

        # ---- conv1 weights (loaded during attention; fp8 hi/lo) ----
        w1all_s = persist.tile([128, 36, C], F8, tag="w1all")
        nc.sync.dma_start(
            out=w1all_s,
            in_=w1all_d.rearrange("s t (a p) o -> p (s t a) o", a=2))
        w1h_s = w1all_s[:, 0:18]
        w1l_s = w1all_s[:, 18:36]

        # ---- conv0: x0'-half naive fp8 DoubleRow on top of the re-injected
        # in0-half partial (identity matmul opens the accumulation) ----
        for ri, (r0, nr) in enumerate(ROW_TILES):
            for oc in range(2):
                pc = psum1.tile([128, 512], F32, name=f"c0ps{ri}_{oc}",
                                tag="ps_acc" if oc == 0 else "ps_cv")
                pcv = pc[:, :nr * W].rearrange("p (r w) -> p r w", w=W)
                which, lr0 = rd(r0)
                ysrc = ybufT if which == "T" else ybufB
                nc.tensor.matmul(pc[:, :nr * W], id_s,
                                 inpart[ri * 2 + oc][:, :nr * W],
                                 start=True, stop=False)
                for t9 in range(9):
                    dh, dw = divmod(t9, 3)
                    nc.tensor.matmul(
                        pcv,
                        w0x_s[:, t9 * 2:t9 * 2 + 2, oc * 128:(oc + 1) * 128],
                        ysrc[:, :, lr0 + dh - 1:lr0 + dh - 1 + nr, dw:dw + W],
                        start=False, stop=(t9 == 8),
                        perf_mode=DR)
                nc.scalar.activation(
                    c0f[:, oc, r0 - 1:r0 - 1 + nr, :], pcv,
                    AF.Gelu, bias=b0_s[:, oc:oc + 1], scale=a0_s[:, oc:oc + 1])
                # hi/lo split of c0 for conv1's 3-term product (DVE), into
                # the top/bottom tiles (boundary rows land in both)
                for w2, lr2, n2, src2 in split_rows(r0, nr):
                    ch = c0hT if w2 == "T" else c0hB
                    cl = c0lT if w2 == "T" else c0lB
                    s2 = r0 - 1 + src2
                    nc.vector.tensor_copy(ch[:, oc, lr2:lr2 + n2, 1:65],
                                          c0f[:, oc, s2:s2 + n2, :])
                    nc.vector.tensor_sub(cl[:, oc, lr2:lr2 + n2, 1:65],
                                         c0f[:, oc, s2:s2 + n2, :],
                                         ch[:, oc, lr2:lr2 + n2, 1:65])

        # ---- conv1: 256 -> 256, 3-term DoubleRow fp8, bn + gelu,
        #      + x0' residual, row-max; per-row-tile output DMA so only the
        #      small last tile sits on the kernel tail ----
        for ri, (r0, nr) in enumerate(ROW_TILES):
            for oc in range(2):
                pc = psum1.tile([128, 512], F32, name=f"c1ps{ri}_{oc}",
                                tag="ps_acc" if oc == 0 else "ps_cv")
                pcv = pc[:, :nr * W].rearrange("p (r w) -> p r w", w=W)
                which, lr0 = rd(r0)
                ch = c0hT if which == "T" else c0hB
                cl = c0lT if which == "T" else c0lB
                terms1 = [(w1h_s, ch), (w1h_s, cl), (w1l_s, ch)]
                i_mm, n_mm = 0, 9 * len(terms1)
                for w_s, x_s in terms1:
                    for t9 in range(9):
                        dh, dw = divmod(t9, 3)
                        nc.tensor.matmul(
                            pcv,
                            w_s[:, t9 * 2:t9 * 2 + 2, oc * 128:(oc + 1) * 128],
                            x_s[:, :, lr0 + dh - 1:lr0 + dh - 1 + nr,
                                dw:dw + W],
                            start=(i_mm == 0), stop=(i_mm == n_mm - 1),
                            perf_mode=DR)
                        i_mm += 1
                tmp = small.tile([128, 512], F32, tag="scratch")
                nc.scalar.activation(tmp[:, :nr * W], pc[:, :nr * W], AF.Gelu,
                                     bias=b1_s[:, oc:oc + 1],
                                     scale=a1_s[:, oc:oc + 1])
                yres = ybufT if r0 + nr <= TOPR else ybufB
                yr0 = r0 if r0 + nr <= TOPR else r0 - BOTR0
                res = small.tile([128, 512], F32, tag="scratch")
                nc.vector.tensor_add(
                    res[:, :nr * W].rearrange("p (r w) -> p r w", w=W),
                    tmp[:, :nr * W].rearrange("p (r w) -> p r w", w=W),
                    yres[:, oc, yr0:yr0 + nr, 1:65])
                outr = small.tile([128, 8], F32, tag="outr")
                nc.vector.reduce_max(
                    outr[:, :nr],
                    res[:, :nr * W].rearrange("p (r w) -> p r w", w=W),
                    axis=AX.X)
                # SP queue only: ACT dispatches would delay the last gelus
                nc.sync.dma_start(
                    out=out[oc * 128:(oc + 1) * 128, r0 - 1:r0 - 1 + nr],
                    in_=outr[:, :nr])

    nc.compile()
    return nc


def _prep_maps(inputs):
    """Host-side input prep: slicing, transposes, BN folding, fp8 splits."""
    f = np.float32
    in0 = np.ascontiguousarray(np.asarray(inputs["inputs_0"], f).reshape(B, C, N))
    in1 = np.ascontiguousarray(np.asarray(inputs["inputs_1"], f).reshape(B, C, N))
    g = {k: np.asarray(v, f) for k, v in inputs.items()}

    def fold(gm, bt, m, v, conv_b):
        a = (gm / np.sqrt(v + EPS)).astype(f)
        return a, (bt - m * a + a * conv_b).astype(f)

    a_bn, b_bn = fold(g["bn0_g"], g["bn0_b"], g["bn0_m"], g["bn0_v"],
                      g["b_o"] + g["w_o"] @ g["b_q"])
    a0, b0 = fold(g["cb_bn0_g"], g["cb_bn0_b"], g["cb_bn0_m"], g["cb_bn0_v"],
                  g["cb_b0"])
    a1, b1 = fold(g["cb_bn1_g"], g["cb_bn1_b"], g["cb_bn1_m"], g["cb_bn1_v"],
                  g["cb_b1"])

    def wsplit(w):
        wh = w.astype(F8NP)
        wl = (w - wh.astype(f)).astype(F8NP)
        return wh, wl

    # conv weights as (tap, ci, o); x0-half naive fp8, in0-half + w1 hi/lo
    w0t = np.ascontiguousarray(
        g["cb_w0"].transpose(2, 3, 1, 0).reshape(9, 2 * C, C))
    w1t = np.ascontiguousarray(
        g["cb_w1"].transpose(2, 3, 1, 0).reshape(9, C, C))
    w0inh, w0inl = wsplit(w0t[:, C:, :])
    w1h, w1l = wsplit(w1t)

    shared = {
        "wq_t": np.ascontiguousarray(g["w_q"].T).astype(BF16NP),
        "wk_t": np.ascontiguousarray(g["w_k"].T).astype(BF16NP),
        "wv_t": np.ascontiguousarray(g["w_v"].T).astype(BF16NP),
        "wo_t": np.ascontiguousarray(g["w_o"].T).astype(BF16NP),
        "bv": np.ascontiguousarray(g["b_v"].reshape(Ch, 1)),
        "bias6": np.ascontiguousarray(
            np.stack([a_bn, b_bn, a0, b0, a1, b1]).reshape(6, C, 1)),
        "w0all": np.ascontiguousarray(
            np.stack([w0t[:, :C, :].astype(F8NP), w0inh, w0inl])),
        "w1all": np.ascontiguousarray(np.stack([w1h, w1l])),
        "idm": np.eye(128, dtype=BF16NP),
    }
    maps = []
    for b in range(B):
        in0b16 = in0[b].astype(BF16NP)
        for half in range(2):
            w0r = 0 if half == 0 else 30
            sl = slice(w0r * W, (w0r + ROWS) * W)
            in0w_f32 = in0[b][:, sl].reshape(C, ROWS, W)
            in0h = np.zeros((C, ROWS, 66), F8NP)
            in0l = np.zeros((C, ROWS, 66), F8NP)
            in0h[:, :, 1:65] = in0w_f32.astype(F8NP)
            in0l[:, :, 1:65] = (
                in0w_f32 - in0h[:, :, 1:65].astype(f)).astype(F8NP)
            maps.append({
                "in0b": in0b16,
                "in0h": in0h.reshape(C, ROWS * 66),
                "in0l": in0l.reshape(C, ROWS * 66),
                "in1b": np.ascontiguousarray(in1[b][:, sl]).astype(BF16NP),
                **shared,
            })
    return maps


def kernel(**inputs):
    if "nc" not in _CACHED:
        _CACHED["nc"] = build_program()
    nc = _CACHED["nc"]
    maps = _prep_maps(inputs)
    res = run_bass_kernel_spmd(nc, maps, core_ids=list(range(8)))
    out = np.zeros((B, C), np.float32)
    for b in range(B):
        top = res.results[2 * b]["out"][:, 0:32].max(axis=1)
        bot = res.results[2 * b + 1]["out"][:, 2:34].max(axis=1)
        out[b] = np.maximum(out[b], np.maximum(top, bot))
    return out


# revision 72
# speedup vs baseline: 1.0290x; 1.0136x over previous
"""Trainium2 Bass kernel for nn_CFAConv (cross-feature attention + conv block).

Self-contained: takes full unsharded inputs, shards (batch, image-half) across
8 NeuronCores, runs one SPMD Bass/Tile NEFF, and combines partial results on
the host.

Math (validated against the jax reference in numpy):
  x1 = w_q@in0 + b_q ; x2 = w_k@in0 + b_k ; x3 = w_v@in1 + b_v  (1x1 convs)
  aff = softmax_j(x2^T x3) ; x0 = x1 @ aff
  x0' = gelu(bn0(w_o@x0 + b_o))
  y = gelu(bn(conv3x3(concat(x0', in0)))) ; y = gelu(bn(conv3x3(y)))
  out = max_spatial(y + x0')
On-device simplifications:
  - softmax over j is invariant to per-column shifts => b_k drops entirely
  - x2^T(x3 + b_v) = x2^T x3 + (x2^T b_v) 1^T    => fold b_v into x3
  - (x1 + b_q 1^T) @ aff = x1@aff + b_q 1^T (aff columns sum to 1)
    => fold w_o@b_q into the out-projection bias (host-side)
  - eval-mode BN folds to per-channel scale/bias, fused into the gelu ACT op
  - softmax normalization deferred past the x1@exp(S) matmul (divide x0 by
    column sums); sums via a 5-level bf16 DVE pre-sum tree + one ones-matmul
  - no max-subtraction in softmax: |S| <= ~60 here; exp fits fp32 (max ~e88)
Precision: bf16 operands with fp32 PSUM accumulation for the attention path;
the two 3x3 convs run in fp8e4m3 with DoubleRow perf mode (2 contraction
tiles per pass at 0.5 cycles/row):
  - conv0 x0'-half: weights + acts naive fp8 (x0' is small vs in0 => cheap)
  - conv0 in0-half: weights hi+lo fp8 split, in0 hi+lo fp8 split (host-side),
    3-term product (Wh Xh + Wh Xl + Wl Xh)
  - conv1: weights hi+lo (host), c0 hi+lo split on DVE, 3-term
  (numpy bit-model: 1.3e-2 final rel err vs the 2e-2 budget)
Sharding: 8 cores = (4 batches) x (top/bottom image half). Each core computes
a 34-row window (32 owned + halo) so the two 3x3 convs need no communication;
per-row maxes [256, 34] go to the host which slices owned rows and reduces.
"""

from contextlib import ExitStack

import ml_dtypes
import numpy as np

import concourse.bass as bass
import concourse.tile as tile
from concourse import bacc, mybir
from concourse.bass_utils import run_bass_kernel_spmd

B, C, H, W = 4, 256, 64, 64
Ch = C // 2          # 128
N = H * W            # 4096
ROWS = 34            # per-core row window (32 owned + 2 halo)
KW = ROWS * W        # 2176 window positions
EPS = 1e-5

F32 = mybir.dt.float32
BF16 = mybir.dt.bfloat16
F8 = mybir.dt.float8e4
AF = mybir.ActivationFunctionType
AX = mybir.AxisListType
DR = mybir.MatmulPerfMode.DoubleRow
BF16NP = ml_dtypes.bfloat16
F8NP = ml_dtypes.float8_e4m3

# attention k-tiles over the 2176-column window
K_TILES = [(0, 512), (512, 512), (1024, 512), (1536, 512), (2048, 128)]
# conv output row-tiles (local rows 1..34 of the 36-row padded buffer)
ROW_TILES = [(1, 8), (9, 8), (17, 8), (25, 8), (33, 2)]

_CACHED = {}


def build_program():
    nc = bacc.Bacc("TRN2", target_bir_lowering=False, debug=False)

    def din(name, shape, dt=F32):
        return nc.dram_tensor(name, shape, dt, kind="ExternalInput").ap()

    in0b_d = din("in0b", [C, N], BF16)
    in1b_d = din("in1b", [C, KW], BF16)
    # in0 conv window, fp8 hi/lo, pre-padded to 66 cols (zero side columns)
    in0h_d = din("in0h", [C, ROWS * 66], F8)
    in0l_d = din("in0l", [C, ROWS * 66], F8)
    wq_t = din("wq_t", [C, Ch], BF16)     # (c, i)
    wk_t = din("wk_t", [C, Ch], BF16)
    wv_t = din("wv_t", [C, Ch], BF16)
    wo_t = din("wo_t", [Ch, C], BF16)     # (i, o)
    bv = din("bv", [Ch, 1])
    bias6_d = din("bias6", [6, C, 1])     # ao, bo, a0, b0, a1, b1
    # conv0 weights: [x0-half naive, in0-half hi, in0-half lo] (tap, ci, o)
    w0all_d = din("w0all", [3, 9, C, C], F8)
    w1all_d = din("w1all", [2, 9, C, C], F8)  # [hi, lo]
    idm_d = din("idm", [128, 128], BF16)
    out = nc.dram_tensor("out", [C, ROWS], F32, kind="ExternalOutput").ap()

    with tile.TileContext(nc) as tc, ExitStack() as ctx:
        persist = ctx.enter_context(tc.tile_pool(name="persist", bufs=1))
        psum = ctx.enter_context(tc.tile_pool(name="psum", bufs=2, space="PSUM"))
        psum1 = ctx.enter_context(tc.tile_pool(name="psum1", bufs=2, space="PSUM"))
        small = ctx.enter_context(tc.tile_pool(name="small", bufs=3))

        # ---- inputs: bf16 quarters of in0 (one DMA each: per-slice deps
        # because Tile dependencies are whole-tile). Weights + a small first
        # slice of in0 go first so the first matmul starts ASAP. ----
        # DMA issue costs ~1.26us/queue: strictly alternate the projection
        # inputs across the SP and ACT HWDGE queues in consumption order
        in0ap = in0b_d.rearrange("(a p) n -> p a n", a=2)
        in0q0a = persist.tile([128, 2, 128], BF16, tag="in0q0a")
        nc.scalar.dma_start(out=in0q0a, in_=in0ap[:, :, 0:128])
        wk_s = persist.tile([128, 2, Ch], BF16, tag="wk")
        nc.sync.dma_start(out=wk_s, in_=wk_t.rearrange("(a p) n -> p a n", a=2))
        in0q0b = persist.tile([128, 2, 384], BF16, tag="in0q0b")
        nc.scalar.dma_start(out=in0q0b, in_=in0ap[:, :, 128:512])
        wq_s = persist.tile([128, 2, Ch], BF16, tag="wq")
        nc.sync.dma_start(out=wq_s, in_=wq_t.rearrange("(a p) n -> p a n", a=2))
        in0q = [None] + [persist.tile([128, 2, 512], BF16, tag=f"in0q{q}",
                                      name=f"in0q{q}") for q in range(1, 8)]
        in1q = [persist.tile([128, 2, 1152], BF16, tag=f"in1q{q}",
                             name=f"in1q{q}") for q in range(2)]
        for q in range(1, 8):
            eng = nc.scalar if q % 2 == 0 else nc.sync
            eng.dma_start(out=in0q[q], in_=in0ap[:, :, q * 512:(q + 1) * 512])
        nc.sync.dma_start(
            out=in1q[0][:, :, :1024],
            in_=in1b_d.rearrange("(a p) n -> p a n", a=2)[:, :, 0:1024])
        nc.scalar.dma_start(
            out=in1q[1],
            in_=in1b_d.rearrange("(a p) n -> p a n", a=2)[:, :, 1024:KW])
        wv_s = persist.tile([128, 2, Ch], BF16, tag="wv")
        nc.scalar.dma_start(out=wv_s,
                            in_=wv_t.rearrange("(a p) n -> p a n", a=2))
        bv_s = persist.tile([128, 1], F32, tag="bv")
        nc.scalar.dma_start(out=bv_s, in_=bv)
        wo_s = persist.tile([128, C], BF16, tag="wo")
        nc.sync.dma_start(out=wo_s, in_=wo_t)
        bias_s = persist.tile([128, 12], F32, tag="bias6")
        nc.sync.dma_start(out=bias_s,
                          in_=bias6_d.rearrange("t (a p) o -> p (t a o)", a=2))
        ao_s, bo_s = bias_s[:, 0:2], bias_s[:, 2:4]
        a0_s, b0_s = bias_s[:, 4:6], bias_s[:, 6:8]
        a1_s, b1_s = bias_s[:, 8:10], bias_s[:, 10:12]
        ones_s = persist.tile([128, 1], BF16, tag="ones")
        nc.vector.memset(ones_s, 1.0)
        ones_row = persist.tile([1, 128], BF16, tag="ones_row")
        nc.vector.memset(ones_row, 1.0)
        id_s = persist.tile([128, 128], BF16, tag="idm")
        nc.sync.dma_start(out=id_s, in_=idm_d)

        # p-state warmup: the PE ramps to full clock only after 3us of
        # continuous execution; spin it on throwaway matmuls while the first
        # input DMAs are in flight so the projections run at full rate
        spin_src = persist.tile([128, 256], BF16, tag="spin_src")
        nc.vector.memset(spin_src, 1.0)
        spin_ps = psum1.tile([128, 512], F32, tag="ps_cv", name="spin_ps")
        for _ in range(7):
            nc.tensor.matmul(spin_ps[0:1, :256], ones_s, spin_src,
                             start=True, stop=True)
        spin_sink = small.tile([1, 16], F32, tag="spin_sink")
        nc.vector.tensor_copy(spin_sink, spin_ps[0:1, :16])  # ring consumer

        # ---- projections: x2 [ch, N], x1T [j, i], x3 [ch, KW] (all bf16) --
        x2_s = persist.tile([128, N], BF16, tag="x2")
        x1t_s = persist.tile([128, 32, Ch], BF16, tag="x1t")
        x3_s = persist.tile([128, KW], BF16, tag="x3")

        def jc_slices(jc, lo, hi):
            """moving-operand slices [lo:hi) of in0 quarter jc (jc 0 is split
            into a 128-col head so the first matmul follows a small DMA)."""
            if jc > 0:
                return [(in0q[jc], lo, hi, lo)]
            out = []
            if lo < 128:
                out.append((in0q0a, lo, min(hi, 128), lo))
            if hi > 128:
                out.append((in0q0b, max(lo, 128) - 128, hi - 128,
                            max(lo, 128)))
            return out

        for jc in range(8):
            # x2 chunk: one accumulation group in one PSUM bank
            ps2 = psum.tile([128, 2, 512], F32, tag="ps_S")
            mms = [(t, a, b, o, cc) for cc in range(2)
                   for (t, a, b, o) in jc_slices(jc, 0, 512)]
            for i, (t, a, b, o, cc) in enumerate(mms):
                nc.tensor.matmul(ps2[:, 0, o:o + b - a], wk_s[:, cc, :],
                                 t[:, cc, a:b],
                                 start=(i == 0), stop=(i == len(mms) - 1))
            nc.vector.tensor_copy(x2_s[:, jc * 512:(jc + 1) * 512], ps2[:, 0, :])
            # x1t: all 4 j-subchunks in ONE group/bank, evicted in one ACT copy
            ps1 = psum1.tile([128, 512], F32, tag="ps_acc")
            mms = [(js, t, a, b, cc) for js in range(4) for cc in range(2)
                   for (t, a, b, _) in jc_slices(jc, js * 128, (js + 1) * 128)]
            for i, (js, t, a, b, cc) in enumerate(mms):
                nc.tensor.matmul(
                    ps1[:, js * 128:js * 128 + Ch],
                    t[:, cc, a:b], wq_s[:, cc, :],
                    start=(i == 0), stop=(i == len(mms) - 1))
            # x1t eviction: DVE while ACT's queue is still dispatching the
            # early input DMAs (~6.3us), ACT once it has drained
            if jc < 5:
                nc.vector.tensor_copy(
                    x1t_s[:, jc * 4:(jc + 1) * 4, :],
                    ps1.rearrange("p (a c) -> p a c", c=Ch))
            else:
                nc.scalar.activation(
                    x1t_s[:, jc * 4:(jc + 1) * 4, :],
                    ps1.rearrange("p (a c) -> p a c", c=Ch), AF.Copy)

        for k0, ksz in K_TILES:
            iq, off = (0, k0) if k0 < 1024 else (1, k0 - 1024)
            ps3 = psum.tile([128, 2, 512], F32, tag="ps_S")
            for cc in range(2):
                nc.tensor.matmul(ps3[:, 0, :ksz], wv_s[:, cc, :],
                                 in1q[iq][:, cc, off:off + ksz],
                                 start=(cc == 0), stop=(cc == 1))
            # x3 = psum + b_v : folds the v-bias into the affinity logits
            nc.vector.tensor_scalar_add(x3_s[:, k0:k0 + ksz], ps3[:, 0, :ksz],
                                        bv_s)

        # ---- conv buffers (fp8, padded 66-wide rows with zero ring) ----
        # ybuf and c0 are split into top (rows 0..18) / bottom (rows 16..35)
        # tiles: conv row-tiles 0-1 read only top, 2-4 only bottom, so each
        # conv stage starts as soon as its first producer rows land instead
        # of waiting for the whole buffer (Tile deps are whole-tile).
        TOPR, BOTR0 = 19, 16  # top rows [0,19), bottom rows [16,36)
        convbuf = ctx.enter_context(tc.tile_pool(name="convbuf", bufs=1))
        ybufT = convbuf.tile([128, 2, TOPR, 66], F8, tag="ybufT")
        ybufB = convbuf.tile([128, 2, 36 - BOTR0, 66], F8, tag="ybufB")
        in0h_s = convbuf.tile([128, 2, 36, 66], F8, tag="in0h")
        in0l_s = convbuf.tile([128, 2, 36, 66], F8, tag="in0l")
        c0hT = convbuf.tile([128, 2, TOPR, 66], F8, tag="c0hT")
        c0hB = convbuf.tile([128, 2, 36 - BOTR0, 66], F8, tag="c0hB")
        c0lT = convbuf.tile([128, 2, TOPR, 66], F8, tag="c0lT")
        c0lB = convbuf.tile([128, 2, 36 - BOTR0, 66], F8, tag="c0lB")
        c0f = convbuf.tile([128, 2, ROWS, W], BF16, tag="c0f")
        for tl in (in0h_s, in0l_s):
            # zero pad rows on the otherwise-idle Pool engine (write-only
            # memset; reading uninitialized SBUF can produce NaNs); side
            # columns arrive zero-padded via the DMA
            nc.gpsimd.memset(tl[:, :, 0, :], 0.0)
            nc.gpsimd.memset(tl[:, :, 35, :], 0.0)
        for tl in (ybufT, c0hT, c0lT):
            nc.gpsimd.memset(tl[:, :, 0, :], 0.0)
            nc.gpsimd.memset(tl[:, :, 1:, 0:1], 0.0)
            nc.gpsimd.memset(tl[:, :, 1:, 65:66], 0.0)
        for tl in (ybufB, c0hB, c0lB):
            nc.gpsimd.memset(tl[:, :, 36 - BOTR0 - 1, :], 0.0)
            nc.gpsimd.memset(tl[:, :, :36 - BOTR0 - 1, 0:1], 0.0)
            nc.gpsimd.memset(tl[:, :, :36 - BOTR0 - 1, 65:66], 0.0)

        def split_rows(r0, nrows):
            """Global padded rows [r0, r0+nrows) -> [(tile, local_r0, n,
            src_off)] covering top and bottom tiles (overlap rows go to
            both)."""
            parts = []
            t_hi = min(r0 + nrows, TOPR)
            if r0 < TOPR:
                parts.append(("T", r0, t_hi - r0, 0))
            b_lo = max(r0, BOTR0)
            if r0 + nrows > BOTR0:
                parts.append(("B", b_lo - BOTR0, r0 + nrows - b_lo,
                              b_lo - r0))
            return parts

        def rd(r0):
            """conv read window [r0-1, r0-1+nr+2) maps to exactly one tile"""
            return ("T", r0) if r0 < 16 else ("B", r0 - BOTR0)
        # in0 conv window ships as fp8 hi/lo straight into the padded tiles
        # (host pre-pads the 66-col side ring so the DMA stays 3-dim)
        nc.sync.dma_start(
            out=in0h_s[:, :, 1:35, :],
            in_=in0h_d.rearrange("(a p) n -> p a n", a=2))
        nc.sync.dma_start(
            out=in0l_s[:, :, 1:35, :],
            in_=in0l_d.rearrange("(a p) n -> p a n", a=2))

        # ---- conv0 weights (early: the in0-half partial sums run inside the
        # attention phase to fill PE slack while ACT grinds the exps) ----
        w0all_s = persist.tile([128, 54, C], F8, tag="w0all")
        nc.sync.dma_start(
            out=w0all_s,
            in_=w0all_d.rearrange("s t (a p) o -> p (s t a) o", a=2))
        w0x_s = w0all_s[:, 0:18]
        w0inh_s = w0all_s[:, 18:36]
        w0inl_s = w0all_s[:, 36:54]
        inpart = [persist.tile([128, 512], BF16, tag=f"inpart{g}",
                               name=f"inpart{g}") for g in range(10)]
        terms_in0 = [(w0inh_s, in0h_s), (w0inh_s, in0l_s), (w0inl_s, in0h_s)]

        def emit_in0_partial(g, tag="ps_cv"):
            """27 DoubleRow passes of conv0's in0-half for group g=(rt,oc),
            evicted to SBUF bf16 for later re-injection."""
            (r0, nr), oc = ROW_TILES[g // 2], g % 2
            ps = psum1.tile([128, 512], F32, tag=tag, name=f"cv{g}")
            pcv = ps[:, :nr * W].rearrange("p (r w) -> p r w", w=W)
            i_mm, n_mm = 0, 27
            for w_s, x_s in terms_in0:
                for t9 in range(9):
                    dh, dw = divmod(t9, 3)
                    nc.tensor.matmul(
                        pcv,
                        w_s[:, t9 * 2:t9 * 2 + 2, oc * 128:(oc + 1) * 128],
                        x_s[:, :, r0 + dh - 1:r0 + dh - 1 + nr, dw:dw + W],
                        start=(i_mm == 0), stop=(i_mm == n_mm - 1),
                        perf_mode=DR)
                    i_mm += 1
            nc.vector.tensor_copy(inpart[g][:, :nr * W], ps[:, :nr * W])

        # ---- attention: S = x2^T x3, exp, x0 = x1 @ exp, sums, normalize ----
        attn = ctx.enter_context(tc.tile_pool(name="attn", bufs=4))
        attn2 = ctx.enter_context(tc.tile_pool(name="attn2", bufs=2))
        dram = ctx.enter_context(tc.tile_pool(name="dram", bufs=5, space="DRAM"))
        # per-k-tile x0n tiles: whole-tile deps would otherwise make each
        # out-projection wait for the LAST k-tile's normalization
        x0n_t = [persist.tile([128, 512], BF16, tag=f"x0n{kt}",
                              name=f"x0n{kt}") for kt in range(len(K_TILES))]
        for kt, (k0, ksz) in enumerate(K_TILES):
            # four quarter-tiles under one bufs=4 tag: stage-2 consumes a
            # quarter while later quarters' exps still run, and the next
            # k-tile's exps begin as soon as a quarter is drained
            expS_h = [attn.tile([128, 8, 512], BF16, tag="expS",
                                name=f"expS{k0}_{h}") for h in range(4)]
            # ssum shares the ps_cv ring with the conv0 in0-half partials:
            # both have fast consumers so the rotation never stalls the PE
            ssum_t = psum1.tile([128, 512], F32, tag="ps_cv",
                                name=f"ssum{k0}")
            for mh in range(16):  # chunk pairs
                sp = psum.tile([128, 2, 512], F32, tag="ps_S")
                for i in range(2):
                    m = 2 * mh + i
                    nc.tensor.matmul(
                        sp[:, i, :ksz],
                        x2_s[:, m * 128:(m + 1) * 128],
                        x3_s[:, k0:k0 + ksz],
                        start=True, stop=True)
                eh = expS_h[mh // 4]
                nc.scalar.activation(
                    eh[:, (2 * mh) % 8:(2 * mh) % 8 + 2, :ksz],
                    sp[:, :, :ksz], AF.Exp)
            # 5-level bf16 pre-sum tree on DVE collapses the softmax
            # column-sum to ONE ones-matmul pass (sum error ~0.3%, only
            # scales the normalization)
            octs = attn2.tile([128, 4, 512], BF16, tag="oct")
            for h in range(4):
                pair = attn.tile([128, 4, 512], BF16, tag="pair",
                                 name=f"pair{k0}_{h}")
                for i in range(4):
                    nc.vector.tensor_add(pair[:, i, :ksz],
                                         expS_h[h][:, 2 * i, :ksz],
                                         expS_h[h][:, 2 * i + 1, :ksz])
                quad = attn.tile([128, 2, 512], BF16, tag="quad",
                                 name=f"quad{k0}_{h}")
                for i in range(2):
                    nc.vector.tensor_add(quad[:, i, :ksz],
                                         pair[:, 2 * i, :ksz],
                                         pair[:, 2 * i + 1, :ksz])
                nc.vector.tensor_add(octs[:, h, :ksz], quad[:, 0, :ksz],
                                     quad[:, 1, :ksz])
            hexs = attn2.tile([128, 2, 512], BF16, tag="hex")
            for i in range(2):
                nc.vector.tensor_add(hexs[:, i, :ksz], octs[:, 2 * i, :ksz],
                                     octs[:, 2 * i + 1, :ksz])
            top = attn2.tile([128, 512], BF16, tag="top")
            nc.vector.tensor_add(top[:, :ksz], hexs[:, 0, :ksz],
                                 hexs[:, 1, :ksz])
            x0p = psum1.tile([128, 512], F32, tag="ps_acc")
            ssum = ssum_t[0:1, :]
            for m in range(32):
                eSm = expS_h[m // 8][:, m % 8, :ksz]
                nc.tensor.matmul(x0p[:, :ksz], x1t_s[:, m, :], eSm,
                                 start=(m == 0), stop=(m == 31))
            # one conv0 in0-half partial fills the tree's DVE latency and
            # balances attention PE load vs the exps; its sibling runs in
            # the ACT-bound out-projection phase instead
            emit_in0_partial(2 * kt)
            if kt == 4:
                # one more bridges the last k-tile's serial
                # tree->ssum->recip->broadcast tail
                emit_in0_partial(1, tag="ps_acc")
            nc.tensor.matmul(ssum[:, :ksz], ones_s, top[:, :ksz],
                             start=True, stop=True)
            sinv = small.tile([1, 512], BF16, tag="sinv")
            with nc.allow_low_precision(
                    reason="bf16 1/colsum only scales the softmax "
                           "normalization; 1.2e-2 in the numpy bit model"):
                nc.vector.reciprocal(sinv[:, :ksz], ssum[:, :ksz])
            # broadcast 1/colsum to all partitions with a 1-row bf16 matmul
            # (a DRAM roundtrip would put ~3us of DMA latency on this chain)
            bcast = psum1.tile([128, 512], F32, tag="ps_cv",
                               name=f"sinv_bcast{kt}")
            nc.tensor.matmul(bcast[:, :ksz], ones_row, sinv[:, :ksz],
                             start=True, stop=True)
            sinvb = small.tile([128, 512], F32, tag="sinvb")
            nc.scalar.activation(sinvb[:, :ksz], bcast[:, :ksz], AF.Copy)
            nc.vector.tensor_mul(x0n_t[kt][:, :ksz], x0p[:, :ksz],
                                 sinvb[:, :ksz])

        # ---- out-projection + bn0 + gelu -> x0' (fp8) into ybuf T/B.
        # Per-k-tile x0n tiles + split ybuf let conv0 start after the first
        # three gelu writes instead of all ten. ----
        for kt, (k0, ksz) in enumerate(K_TILES):
            nr = ksz // W
            for oc in range(2):
                po = psum1.tile([128, 512], F32, name=f"po{kt}_{oc}",
                                tag="ps_acc" if oc == 0 else "ps_cv")
                nc.tensor.matmul(po[:, :ksz],
                                 wo_s[:, oc * 128:(oc + 1) * 128],
                                 x0n_t[kt][:, :ksz],
                                 start=True, stop=True)
                pv = po[:, :ksz].rearrange("p (r w) -> p r w", w=W)
                for which, lr0, n, src in split_rows(1 + kt * 8, nr):
                    dst = ybufT if which == "T" else ybufB
                    nc.scalar.activation(
                        dst[:, oc, lr0:lr0 + n, 1:65], pv[:, src:src + n],
                        AF.Gelu, bias=bo_s[:, oc:oc + 1],
                        scale=ao_s[:, oc:oc + 1])
            # leftover in0-half partials: this phase is ACT-gelu-bound
            # (g1 was pulled into the last attention k-tile's tail bubble)
            if kt >= 1:
                emit_in0_partial(2 * kt + 1, tag="ps_acc")

        # ---- conv1 weights (loaded during attention; fp8 hi/lo) ----
        w1all_s = persist.tile([128, 36, C], F8, tag="w1all")
        nc.sync.dma_start(
            out=w1all_s,
            in_=w1all_d.rearrange("s t (a p) o -> p (s t a) o", a=2))
        w1h_s = w1all_s[:, 0:18]
        w1l_s = w1all_s[:, 18:36]

        # ---- conv0: x0'-half naive fp8 DoubleRow on top of the re-injected
        # in0-half partial (identity matmul opens the accumulation) ----
        for ri, (r0, nr) in enumerate(ROW_TILES):
            for oc in range(2):
                pc = psum1.tile([128, 512], F32, name=f"c0ps{ri}_{oc}",
                                tag="ps_acc" if oc == 0 else "ps_cv")
                pcv = pc[:, :nr * W].rearrange("p (r w) -> p r w", w=W)
                which, lr0 = rd(r0)
                ysrc = ybufT if which == "T" else ybufB
                nc.tensor.matmul(pc[:, :nr * W], id_s,
                                 inpart[ri * 2 + oc][:, :nr * W],
                                 start=True, stop=False)
                for t9 in range(9):
                    dh, dw = divmod(t9, 3)
                    nc.tensor.matmul(
                        pcv,
                        w0x_s[:, t9 * 2:t9 * 2 + 2, oc * 128:(oc + 1) * 128],
                        ysrc[:, :, lr0 + dh - 1:lr0 + dh - 1 + nr, dw:dw + W],
                        start=False, stop=(t9 == 8),
                        perf_mode=DR)
                nc.scalar.activation(
                    c0f[:, oc, r0 - 1:r0 - 1 + nr, :], pcv,
                    AF.Gelu, bias=b0_s[:, oc:oc + 1], scale=a0_s[:, oc:oc + 1])
                # hi/lo split of c0 for conv1's 3-term product (DVE), into
                # the top/bottom tiles (boundary rows land in both)
                for w2, lr2, n2, src2 in split_rows(r0, nr):
                    ch = c0hT if w2 == "T" else c0hB
                    cl = c0lT if w2 == "T" else c0lB
                    s2 = r0 - 1 + src2
                    nc.vector.tensor_copy(ch[:, oc, lr2:lr2 + n2, 1:65],
                                          c0f[:, oc, s2:s2 + n2, :])
                    nc.vector.tensor_sub(cl[:, oc, lr2:lr2 + n2, 1:65],
                                         c0f[:, oc, s2:s2 + n2, :],
                                         ch[:, oc, lr2:lr2 + n2, 1:65])

        # ---- conv1: 256 -> 256, 3-term DoubleRow fp8, bn + gelu,
        #      + x0' residual, row-max; per-row-tile output DMA so only the
        #      small last tile sits on the kernel tail ----
        for ri, (r0, nr) in enumerate(ROW_TILES):
            for oc in range(2):
                pc = psum1.tile([128, 512], F32, name=f"c1ps{ri}_{oc}",
                                tag="ps_acc" if oc == 0 else "ps_cv")
                pcv = pc[:, :nr * W].rearrange("p (r w) -> p r w", w=W)
                which, lr0 = rd(r0)
                ch = c0hT if which == "T" else c0hB
                cl = c0lT if which == "T" else c0lB
                terms1 = [(w1h_s, ch), (w1h_s, cl), (w1l_s, ch)]
                i_mm, n_mm = 0, 9 * len(terms1)
                for w_s, x_s in terms1:
                    for t9 in range(9):
                        dh, dw = divmod(t9, 3)
                        nc.tensor.matmul(
                            pcv,
                            w_s[:, t9 * 2:t9 * 2 + 2, oc * 128:(oc + 1) * 128],
                            x_s[:, :, lr0 + dh - 1:lr0 + dh - 1 + nr,
                                dw:dw + W],
                            start=(i_mm == 0), stop=(i_mm == n_mm - 1),
                            perf_mode=DR)
                        i_mm += 1
                tmp = small.tile([128, 512], F32, tag="scratch")
                nc.scalar.activation(tmp[:, :nr * W], pc[:, :nr * W], AF.Gelu,
                                     bias=b1_s[:, oc:oc + 1],
                                     scale=a1_s[:, oc:oc + 1])
                yres = ybufT if r0 + nr <= TOPR else ybufB
                yr0 = r0 if r0 + nr <= TOPR else r0 - BOTR0
                res = small.tile([128, 512], F32, tag="scratch")
                nc.vector.tensor_add(
                    res[:, :nr * W].rearrange("p (r w) -> p r w", w=W),
                    tmp[:, :nr * W].rearrange("p (r w) -> p r w", w=W),
                    yres[:, oc, yr0:yr0 + nr, 1:65])
                outr = small.tile([128, 8], F32, tag="outr")
                nc.vector.reduce_max(
                    outr[:, :nr],
                    res[:, :nr * W].rearrange("p (r w) -> p r w", w=W),
                    axis=AX.X)
                # SP queue only: ACT dispatches would delay the last gelus
                nc.sync.dma_start(
                    out=out[oc * 128:(oc + 1) * 128, r0 - 1:r0 - 1 + nr],
                    in_=outr[:, :nr])

    nc.compile()
    return nc


def _prep_maps(inputs):
    """Host-side input prep: slicing, transposes, BN folding, fp8 splits."""
    f = np.float32
    in0 = np.ascontiguousarray(np.asarray(inputs["inputs_0"], f).reshape(B, C, N))
    in1 = np.ascontiguousarray(np.asarray(inputs["inputs_1"], f).reshape(B, C, N))
    g = {k: np.asarray(v, f) for k, v in inputs.items()}

    def fold(gm, bt, m, v, conv_b):
        a = (gm / np.sqrt(v + EPS)).astype(f)
        return a, (bt - m * a + a * conv_b).astype(f)

    a_bn, b_bn = fold(g["bn0_g"], g["bn0_b"], g["bn0_m"], g["bn0_v"],
                      g["b_o"] + g["w_o"] @ g["b_q"])
    a0, b0 = fold(g["cb_bn0_g"], g["cb_bn0_b"], g["cb_bn0_m"], g["cb_bn0_v"],
                  g["cb_b0"])
    a1, b1 = fold(g["cb_bn1_g"], g["cb_bn1_b"], g["cb_bn1_m"], g["cb_bn1_v"],
                  g["cb_b1"])

    def wsplit(w):
        wh = w.astype(F8NP)
        wl = (w - wh.astype(f)).astype(F8NP)
        return wh, wl

    # conv weights as (tap, ci, o); x0-half naive fp8, in0-half + w1 hi/lo
    w0t = np.ascontiguousarray(
        g["cb_w0"].transpose(2, 3, 1, 0).reshape(9, 2 * C, C))
    w1t = np.ascontiguousarray(
        g["cb_w1"].transpose(2, 3, 1, 0).reshape(9, C, C))
    w0inh, w0inl = wsplit(w0t[:, C:, :])
    w1h, w1l = wsplit(w1t)

    shared = {
        "wq_t": np.ascontiguousarray(g["w_q"].T).astype(BF16NP),
        "wk_t": np.ascontiguousarray(g["w_k"].T).astype(BF16NP),
        "wv_t": np.ascontiguousarray(g["w_v"].T).astype(BF16NP),
        "wo_t": np.ascontiguousarray(g["w_o"].T).astype(BF16NP),
        "bv": np.ascontiguousarray(g["b_v"].reshape(Ch, 1)),
        "bias6": np.ascontiguousarray(
            np.stack([a_bn, b_bn, a0, b0, a1, b1]).reshape(6, C, 1)),
        "w0all": np.ascontiguousarray(
            np.stack([w0t[:, :C, :].astype(F8NP), w0inh, w0inl])),
        "w1all": np.ascontiguousarray(np.stack([w1h, w1l])),
        "idm": np.eye(128, dtype=BF16NP),
    }
    maps = []
    for b in range(B):
        in0b16 = in0[b].astype(BF16NP)
        for half in range(2):
            w0r = 0 if half == 0 else 30
            sl = slice(w0r * W, (w0r + ROWS) * W)
            in0w_f32 = in0[b][:, sl].reshape(C, ROWS, W)
            in0h = np.zeros((C, ROWS, 66), F8NP)
            in0l = np.zeros((C, ROWS, 66), F8NP)
            in0h[:, :, 1:65] = in0w_f32.astype(F8NP)
            in0l[:, :, 1:65] = (
                in0w_f32 - in0h[:, :, 1:65].astype(f)).astype(F8NP)
            maps.append({
                "in0b": in0b16,
                "in0h": in0h.reshape(C, ROWS * 66),
                "in0l": in0l.reshape(C, ROWS * 66),
                "in1b": np.ascontiguousarray(in1[b][:, sl]).astype(BF16NP),
                **shared,
            })
    return maps


def kernel(**inputs):
    if "nc" not in _CACHED:
        _CACHED["nc"] = build_program()
    nc = _CACHED["nc"]
    maps = _prep_maps(inputs)
    res = run_bass_kernel_spmd(nc, maps, core_ids=list(range(8)))
    out = np.zeros((B, C), np.float32)
    for b in range(B):
        top = res.results[2 * b]["out"][:, 0:32].max(axis=1)
        bot = res.results[2 * b + 1]["out"][:, 2:34].max(axis=1)
        out[b] = np.maximum(out[b], np.maximum(top, bot))
    return out
